# revision 3
# baseline (speedup 1.0000x reference)
"""Causal self-attention (B=2, T=2048, C=1024, H=16) on 8 TRN2 NeuronCores.

Sharding: data-parallel on batch (2) x tensor-parallel on heads (4 groups of
4 heads) = 8 cores. Each core computes, for its batch b and head group g:
  QKV^T projection for its 256 qkv columns, causal flash-style attention for
  its 4 heads, and a partial output projection  Y_g @ W_proj[256g:256(g+1)].
The host sums the 4 partial projections per batch and adds b_proj.

On-chip dataflow (all matmuls in float32r ~ tf32):
  xT   = transpose(x_b)                [C-part, T]      (PE transpose)
  Q^T  = Wq.T @ x via lhsT=Wq chunks   [qcol-part, T]
  K^T  likewise; V natural             [T-part, vcol]
  S^T  = K^T_blk.T @ Q^T               [Tk-part, Tq]  (2 heads row-packed)
  E    = exp(S^T/8) (ACT, PSUM->SBUF), causal mask on diagonal blocks
  Ynum^T, denom = [V_h | 1].T @ E      [65-part, Tq]  (PSUM accumulated)
  Y^T  = Ynum^T * (1/denom)            (GPSIMD bcast + DVE)
  out += Y^T.T @ Wp                    [Tq-part, cout]
"""

import numpy as np

import concourse.bacc as bacc
import concourse.mybir as mybir
from concourse import bass_utils
from concourse.bass import ts
from concourse.masks import make_identity
from concourse.tile import TileContext

P = 128
T = 2048
C = 1024
KO = C // P          # 8 contraction chunks over C
HC = 256             # qkv columns per core (4 heads x 64)
NH = 4               # heads per core
D = 64
NTK = T // P         # 16 key blocks
TQB = 512            # query block (free dim)
NQ = T // TQB        # 4 query blocks
SCALE = 1.0 / np.sqrt(D)

f32 = mybir.dt.float32
f32r = mybir.dt.float32r
AF = mybir.ActivationFunctionType
ALU = mybir.AluOpType

_NC = None


def _build():
    nc = bacc.Bacc(trn_type="TRN2", target_bir_lowering=False, debug=False)

    x_d = nc.dram_tensor("x", [T, C], f32, kind="ExternalInput")
    wq_d = nc.dram_tensor("wq", [C, HC], f32r, kind="ExternalInput")
    wk_d = nc.dram_tensor("wk", [C, HC], f32r, kind="ExternalInput")
    wv_d = nc.dram_tensor("wv", [C, HC], f32r, kind="ExternalInput")
    wp_d = nc.dram_tensor("wp", [HC, C], f32r, kind="ExternalInput")
    bq_d = nc.dram_tensor("bq", [HC], f32, kind="ExternalInput")
    bk_d = nc.dram_tensor("bk", [HC], f32, kind="ExternalInput")
    bv_d = nc.dram_tensor("bv", [HC], f32, kind="ExternalInput")
    out_d = nc.dram_tensor("out", [T, C], f32, kind="ExternalOutput")

    with TileContext(nc) as tc:
        with (
            tc.tile_pool(name="persist", bufs=1) as pp,
            tc.tile_pool(name="psum", bufs=2, space="PSUM") as ps,
        ):
            wq = pp.tile([P, KO, HC], f32r, tag="wq")
            wk = pp.tile([P, KO, HC], f32r, tag="wk")
            wv = pp.tile([P, KO, HC], f32r, tag="wv")
            wp = pp.tile([P, 2, C], f32r, tag="wp")
            bqt = pp.tile([P, 2], f32, tag="bqt")
            bkt = pp.tile([P, 2], f32, tag="bkt")
            bvt = pp.tile([P, HC], f32, tag="bvt")
            qt = pp.tile([P, 2, T], f32r, tag="qt")
            kt = pp.tile([P, 2, T], f32r, tag="kt")
            v = pp.tile([P, NTK, NH, D + 1], f32r, tag="v")
            yt = pp.tile([P, 2, T], f32r, tag="yt")
            mask = pp.tile([P, 896], f32, tag="mask")
            ident = pp.tile([P, P], f32, tag="ident")

            nc.sync.dma_start(wq[:], wq_d.ap().rearrange("(ko p) n -> p ko n", p=P))
            nc.sync.dma_start(wk[:], wk_d.ap().rearrange("(ko p) n -> p ko n", p=P))
            nc.sync.dma_start(wv[:], wv_d.ap().rearrange("(ko p) n -> p ko n", p=P))
            nc.sync.dma_start(wp[:], wp_d.ap().rearrange("(kc p) n -> p kc n", p=P))
            nc.sync.dma_start(bqt[:], bq_d.ap().rearrange("(c p) -> p c", p=P))
            nc.sync.dma_start(bkt[:], bk_d.ap().rearrange("(c p) -> p c", p=P))
            nc.sync.dma_start(bvt[:], bv_d.ap()[None, :].to_broadcast((P, HC)))

            make_identity(nc, ident[:])
            # ones columns for the denominator rows of V_aug
            nc.gpsimd.memset(v[:].bitcast(f32), 1.0)
            # sliding causal mask: mask[p, x] = 1 iff p <= x - 384
            nc.gpsimd.memset(mask[:], 1.0)
            nc.gpsimd.affine_select(
                out=mask[:],
                in_=mask[:],
                compare_op=ALU.is_ge,
                fill=0.0,
                base=-384,
                pattern=[[1, 896]],
                channel_multiplier=-1,
            )

            # ---- Phase A: load x and transpose to xT[C-part, T] ----
            with (
                tc.tile_pool(name="xT", bufs=1) as xtp,
                tc.tile_pool(name="xn", bufs=3) as xnp,
            ):
                xT = xtp.tile([P, KO, T], f32r, tag="xT")
                for ti in range(NTK):
                    xn = xnp.tile([P, C], f32, tag="xn")
                    nc.sync.dma_start(xn[:], x_d.ap()[ts(ti, P), :])
                    for kk in range(0, KO, 4):
                        pt = ps.tile([P, 512], f32, tag="mm512")
                        for j in range(4):
                            nc.tensor.transpose(
                                pt[:, ts(j, P)],
                                xn[:, ts(kk + j, P)],
                                ident[:],
                            )
                        nc.vector.tensor_copy(
                            xT[:, kk : kk + 4, ts(ti, P)],
                            pt[:].rearrange("p (k t) -> p k t", k=4),
                        )

                # ---- Phase B: Q^T, K^T, V ----
                for cc in range(2):
                    for tb in range(NQ):
                        pq = ps.tile([P, TQB], f32, tag="mm512")
                        for ko in range(KO):
                            nc.tensor.matmul(
                                pq[:],
                                wq[:, ko, ts(cc, P)],
                                xT[:, ko, ts(tb, TQB)],
                                start=(ko == 0),
                                stop=(ko == KO - 1),
                            )
                        nc.vector.tensor_scalar_add(
                            qt[:, cc, ts(tb, TQB)], pq[:], bqt[:, cc : cc + 1]
                        )
                        pk = ps.tile([P, TQB], f32, tag="mm512")
                        for ko in range(KO):
                            nc.tensor.matmul(
                                pk[:],
                                wk[:, ko, ts(cc, P)],
                                xT[:, ko, ts(tb, TQB)],
                                start=(ko == 0),
                                stop=(ko == KO - 1),
                            )
                        nc.vector.tensor_scalar_add(
                            kt[:, cc, ts(tb, TQB)], pk[:], bkt[:, cc : cc + 1]
                        )
                for ti in range(NTK):
                    pv = ps.tile([P, HC], f32, tag="mm512")
                    for ko in range(KO):
                        nc.tensor.matmul(
                            pv[:],
                            xT[:, ko, ts(ti, P)],
                            wv[:, ko, :],
                            start=(ko == 0),
                            stop=(ko == KO - 1),
                        )
                    nc.vector.tensor_tensor(
                        v[:, ti, :, 0:D],
                        pv[:].rearrange("p (h a) -> p h a", h=NH),
                        bvt[:].rearrange("p (h a) -> p h a", h=NH),
                        ALU.add,
                    )

            # ---- Phase C: causal attention, 2 heads row-packed ----
            with (
                tc.tile_pool(name="e", bufs=4) as ep,
                tc.tile_pool(name="r", bufs=4) as rp,
                tc.tile_pool(name="o", bufs=3) as op,
            ):
                for hp in range(2):
                    h0, h1 = 2 * hp, 2 * hp + 1
                    for tqb in range(NQ):
                        y0 = ps.tile([D + 1, TQB], f32, tag="y")
                        y1 = ps.tile([D + 1, TQB], f32, tag="y")
                        q0 = qt[0:64, hp, ts(tqb, TQB)]
                        q1 = qt[64:128, hp, ts(tqb, TQB)]
                        ntk = 4 * (tqb + 1)
                        for tkk in range(0, ntk, 2):
                            s0 = ps.tile([P, 2 * TQB], f32, tag="s")
                            s1 = ps.tile([P, 2 * TQB], f32, tag="s")
                            for j in range(2):
                                tk = tkk + j
                                nc.tensor.matmul(
                                    s0[:, ts(j, TQB)],
                                    kt[0:64, hp, ts(tk, P)],
                                    q0,
                                    start=True,
                                    stop=True,
                                    tile_position=(0, 0),
                                )
                                nc.tensor.matmul(
                                    s1[:, ts(j, TQB)],
                                    kt[64:128, hp, ts(tk, P)],
                                    q1,
                                    start=True,
                                    stop=True,
                                    tile_position=(64, 0),
                                )
                            e0 = ep.tile([P, 2 * TQB], f32r, tag="e")
                            e1 = ep.tile([P, 2 * TQB], f32r, tag="e")
                            nc.scalar.activation(e0[:], s0[:], AF.Exp, scale=SCALE)
                            nc.scalar.activation(e1[:], s1[:], AF.Exp, scale=SCALE)
                            for j in range(2):
                                tk = tkk + j
                                if tk >= 4 * tqb:
                                    off = 384 + TQB * tqb - P * tk
                                    ms = mask[:, off : off + TQB].bitcast(f32r)
                                    nc.vector.tensor_mul(
                                        e0[:, ts(j, TQB)], e0[:, ts(j, TQB)], ms
                                    )
                                    nc.vector.tensor_mul(
                                        e1[:, ts(j, TQB)], e1[:, ts(j, TQB)], ms
                                    )
                            for j in range(2):
                                tk = tkk + j
                                nc.tensor.matmul(
                                    y0[:],
                                    v[:, tk, h0, :],
                                    e0[:, ts(j, TQB)],
                                    start=(tk == 0),
                                    stop=(tk == ntk - 1),
                                )
                                nc.tensor.matmul(
                                    y1[:],
                                    v[:, tk, h1, :],
                                    e1[:, ts(j, TQB)],
                                    start=(tk == 0),
                                    stop=(tk == ntk - 1),
                                )
                        rec0 = rp.tile([1, TQB], f32, tag="rec")
                        rec1 = rp.tile([1, TQB], f32, tag="rec")
                        nc.vector.reciprocal(rec0[:], y0[64:65, :])
                        nc.vector.reciprocal(rec1[:], y1[64:65, :])
                        rb0 = rp.tile([D, TQB], f32, tag="rb")
                        rb1 = rp.tile([D, TQB], f32, tag="rb")
                        nc.gpsimd.partition_broadcast(rb0[:], rec0[:])
                        nc.gpsimd.partition_broadcast(rb1[:], rec1[:])
                        nc.vector.tensor_mul(
                            yt[0:64, hp, ts(tqb, TQB)], y0[0:64, :], rb0[:]
                        )
                        nc.vector.tensor_mul(
                            yt[64:128, hp, ts(tqb, TQB)], y1[0:64, :], rb1[:]
                        )

                # ---- Phase D: partial output projection ----
                for mt in range(NTK):
                    for nb in range(2):
                        pj = ps.tile([P, 512], f32, tag="mm512")
                        for kc in range(2):
                            nc.tensor.matmul(
                                pj[:],
                                yt[:, kc, ts(mt, P)],
                                wp[:, kc, ts(nb, 512)],
                                start=(kc == 0),
                                stop=(kc == 1),
                            )
                        ot = op.tile([P, 512], f32, tag="ot")
                        nc.vector.tensor_copy(ot[:], pj[:])
                        nc.sync.dma_start(out_d.ap()[ts(mt, P), ts(nb, 512)], ot[:])

    nc.compile()
    return nc


def _get_nc():
    global _NC
    if _NC is None:
        _NC = _build()
    return _NC


def _shard(x, W_qkv, b_qkv, W_proj, b_proj):
    x = np.ascontiguousarray(np.asarray(x, dtype=np.float32))
    W_qkv = np.ascontiguousarray(np.asarray(W_qkv, dtype=np.float32))
    b_qkv = np.ascontiguousarray(np.asarray(b_qkv, dtype=np.float32))
    W_proj = np.ascontiguousarray(np.asarray(W_proj, dtype=np.float32))
    in_maps = []
    for core in range(8):
        b, g = core // 4, core % 4
        cs = slice(g * HC, (g + 1) * HC)
        in_maps.append(
            {
                "x": np.ascontiguousarray(x[b]),
                "wq": np.ascontiguousarray(W_qkv[:, 0 * C :][:, cs]),
                "wk": np.ascontiguousarray(W_qkv[:, 1 * C :][:, cs]),
                "wv": np.ascontiguousarray(W_qkv[:, 2 * C :][:, cs]),
                "wp": np.ascontiguousarray(W_proj[cs, :]),
                "bq": np.ascontiguousarray(b_qkv[0 * C :][cs]),
                "bk": np.ascontiguousarray(b_qkv[1 * C :][cs]),
                "bv": np.ascontiguousarray(b_qkv[2 * C :][cs]),
            }
        )
    return in_maps


def _gather(results, b_proj):
    b_proj = np.asarray(b_proj, dtype=np.float32)
    y = np.empty((2, T, C), dtype=np.float32)
    for b in range(2):
        acc = results[4 * b]["out"].astype(np.float32).copy()
        for g in range(1, 4):
            acc += results[4 * b + g]["out"]
        y[b] = acc + b_proj
    return y


def kernel(x, W_qkv, b_qkv, W_proj, b_proj):
    nc = _get_nc()
    in_maps = _shard(x, W_qkv, b_qkv, W_proj, b_proj)
    res = bass_utils.run_bass_kernel_spmd(nc, in_maps, core_ids=list(range(8)))
    return _gather(res.results, b_proj)


# revision 10
# speedup vs baseline: 1.1532x; 1.1532x over previous
"""Causal self-attention (B=2, T=2048, C=1024, H=16) on 8 TRN2 NeuronCores.

Sharding: data-parallel on batch (2) x tensor-parallel on heads (4 groups of
4 heads) = 8 cores. Each core computes, for its batch b and head group g:
  QKV^T projection for its 256 qkv columns, causal flash-style attention for
  its 4 heads, and a partial output projection  Y_g @ W_proj[256g:256(g+1)].
The host sums the 4 partial projections per batch and adds b_proj.

On-chip dataflow (all matmuls in float32r ~ tf32):
  xT   = transpose(x_b)                [C-part, T]      (PE transpose)
  Q^T  = Wq.T @ x via lhsT=Wq chunks   [qcol-part, T]
  K^T  likewise; V natural             [T-part, vcol]
  S^T  = K^T_blk.T @ Q^T               [Tk-part, Tq]  (2 heads row-packed)
  E    = exp(S^T/8) (ACT, PSUM->SBUF), causal mask on diagonal blocks
  Ynum^T, denom = [V_h | 1].T @ E      [65-part, Tq]  (PSUM accumulated)
  Y^T  = Ynum^T * (1/denom)            (GPSIMD bcast + DVE)
  out += Y^T.T @ Wp                    [Tq-part, cout]
"""

import numpy as np

import concourse.bacc as bacc
import concourse.mybir as mybir
from concourse import bass_utils
from concourse.bass import ts
from concourse.masks import make_identity
from concourse.tile import TileContext

P = 128
T = 2048
C = 1024
KO = C // P          # 8 contraction chunks over C
HC = 256             # qkv columns per core (4 heads x 64)
NH = 4               # heads per core
D = 64
NTK = T // P         # 16 key blocks
TQB = 512            # query block (free dim)
NQ = T // TQB        # 4 query blocks
SCALE = 1.0 / np.sqrt(D)

f32 = mybir.dt.float32
f32r = mybir.dt.float32r
AF = mybir.ActivationFunctionType
ALU = mybir.AluOpType

_NC = None


def _build():
    nc = bacc.Bacc(trn_type="TRN2", target_bir_lowering=False, debug=False)

    x_d = nc.dram_tensor("x", [T, C], f32, kind="ExternalInput")
    wq_d = nc.dram_tensor("wq", [C, HC], f32r, kind="ExternalInput")
    wk_d = nc.dram_tensor("wk", [C, HC], f32r, kind="ExternalInput")
    wv_d = nc.dram_tensor("wv", [C, HC], f32r, kind="ExternalInput")
    wp_d = nc.dram_tensor("wp", [HC, C], f32r, kind="ExternalInput")
    bq_d = nc.dram_tensor("bq", [HC], f32, kind="ExternalInput")
    bk_d = nc.dram_tensor("bk", [HC], f32, kind="ExternalInput")
    bv_d = nc.dram_tensor("bv", [HC], f32, kind="ExternalInput")
    out_d = nc.dram_tensor("out", [T, C], f32, kind="ExternalOutput")

    with TileContext(nc) as tc:
        with (
            tc.tile_pool(name="persist", bufs=1) as pp,
            tc.tile_pool(name="psum", bufs=2, space="PSUM") as ps,
        ):
            wq = pp.tile([P, KO, HC], f32r, tag="wq")
            wk = pp.tile([P, KO, HC], f32r, tag="wk")
            wv = pp.tile([P, KO, HC], f32r, tag="wv")
            wp = pp.tile([P, 2, C], f32r, tag="wp")
            bqt = pp.tile([P, 2], f32, tag="bqt")
            bkt = pp.tile([P, 2], f32, tag="bkt")
            bvt = pp.tile([P, HC], f32, tag="bvt")
            qt = pp.tile([P, 2, T], f32r, tag="qt")
            kt = pp.tile([P, 2, T], f32r, tag="kt")
            v = pp.tile([P, NTK, NH, D + 1], f32r, tag="v")
            yt = pp.tile([P, 2, T], f32r, tag="yt")
            mask = pp.tile([P, P], f32, tag="mask")
            ident = pp.tile([P, P], f32, tag="ident")

            nc.sync.dma_start(wq[:], wq_d.ap().rearrange("(ko p) n -> p ko n", p=P))
            nc.sync.dma_start(wk[:], wk_d.ap().rearrange("(ko p) n -> p ko n", p=P))
            nc.sync.dma_start(wv[:], wv_d.ap().rearrange("(ko p) n -> p ko n", p=P))
            nc.sync.dma_start(wp[:], wp_d.ap().rearrange("(kc p) n -> p kc n", p=P))
            nc.sync.dma_start(bqt[:], bq_d.ap().rearrange("(c p) -> p c", p=P))
            nc.sync.dma_start(bkt[:], bk_d.ap().rearrange("(c p) -> p c", p=P))
            nc.sync.dma_start(bvt[:], bv_d.ap()[None, :].to_broadcast((P, HC)))

            make_identity(nc, ident[:])
            # ones columns for the denominator rows of V_aug
            nc.gpsimd.memset(v[:].bitcast(f32), 1.0)
            # triangle mask: mask[p, f] = 1 iff p <= f
            nc.gpsimd.memset(mask[:], 1.0)
            nc.gpsimd.affine_select(
                out=mask[:],
                in_=mask[:],
                compare_op=ALU.is_ge,
                fill=0.0,
                base=0,
                pattern=[[1, P]],
                channel_multiplier=-1,
            )

            # ---- Phase A: load x and transpose to xT[C-part, T] ----
            with (
                tc.tile_pool(name="xT", bufs=1) as xtp,
                tc.tile_pool(name="xn", bufs=3) as xnp,
            ):
                xT = xtp.tile([P, KO, T], f32r, tag="xT")
                for ti in range(NTK):
                    xn = xnp.tile([P, C], f32, tag="xn")
                    nc.sync.dma_start(xn[:], x_d.ap()[ts(ti, P), :])
                    for kk in range(0, KO, 4):
                        pt = ps.tile([P, 512], f32, tag="mm512")
                        for j in range(4):
                            nc.tensor.transpose(
                                pt[:, ts(j, P)],
                                xn[:, ts(kk + j, P)],
                                ident[:],
                            )
                        nc.scalar.copy(
                            xT[:, kk : kk + 4, ts(ti, P)],
                            pt[:].rearrange("p (k t) -> p k t", k=4),
                        )

                # ---- Phase B: Q^T, K^T, V ----
                for cc in range(2):
                    for tb in range(NQ):
                        pq = ps.tile([P, TQB], f32, tag="mm512")
                        for ko in range(KO):
                            nc.tensor.matmul(
                                pq[:],
                                wq[:, ko, ts(cc, P)],
                                xT[:, ko, ts(tb, TQB)],
                                start=(ko == 0),
                                stop=(ko == KO - 1),
                            )
                        nc.vector.tensor_scalar_add(
                            qt[:, cc, ts(tb, TQB)], pq[:], bqt[:, cc : cc + 1]
                        )
                        pk = ps.tile([P, TQB], f32, tag="mm512")
                        for ko in range(KO):
                            nc.tensor.matmul(
                                pk[:],
                                wk[:, ko, ts(cc, P)],
                                xT[:, ko, ts(tb, TQB)],
                                start=(ko == 0),
                                stop=(ko == KO - 1),
                            )
                        nc.vector.tensor_scalar_add(
                            kt[:, cc, ts(tb, TQB)], pk[:], bkt[:, cc : cc + 1]
                        )
                for ti in range(NTK):
                    pv = ps.tile([P, HC], f32, tag="mm512")
                    for ko in range(KO):
                        nc.tensor.matmul(
                            pv[:],
                            xT[:, ko, ts(ti, P)],
                            wv[:, ko, :],
                            start=(ko == 0),
                            stop=(ko == KO - 1),
                        )
                    nc.vector.tensor_tensor(
                        v[:, ti, :, 0:D],
                        pv[:].rearrange("p (h a) -> p h a", h=NH),
                        bvt[:].rearrange("p (h a) -> p h a", h=NH),
                        ALU.add,
                    )

            # ---- Phase C: causal attention, 2 heads row-packed ----
            with (
                tc.tile_pool(name="e", bufs=6) as ep,
                tc.tile_pool(name="r", bufs=4) as rp,
                tc.tile_pool(name="o", bufs=3) as op,
            ):
                tri = mask[:, 0:P].bitcast(f32r)
                for tqb in range(NQ):
                    for hp in range(2):
                        h0, h1 = 2 * hp, 2 * hp + 1
                        y0 = ps.tile([D + 1, TQB], f32, tag="y")
                        y1 = ps.tile([D + 1, TQB], f32, tag="y")
                        q0 = qt[0:64, hp, ts(tqb, TQB)]
                        q1 = qt[64:128, hp, ts(tqb, TQB)]
                        ntk = 4 * (tqb + 1)
                        for tkk in range(0, ntk, 2):
                            s0 = ps.tile([P, 2 * TQB], f32, tag="s")
                            s1 = ps.tile([P, 2 * TQB], f32, tag="s")
                            for j in range(2):
                                tk = tkk + j
                                nc.tensor.matmul(
                                    s0[:, ts(j, TQB)],
                                    kt[0:64, hp, ts(tk, P)],
                                    q0,
                                    start=True,
                                    stop=True,
                                    tile_position=(0, 0),
                                )
                                nc.tensor.matmul(
                                    s1[:, ts(j, TQB)],
                                    kt[64:128, hp, ts(tk, P)],
                                    q1,
                                    start=True,
                                    stop=True,
                                    tile_position=(64, 0),
                                )
                            e0 = ep.tile([P, 2 * TQB], f32r, tag="e")
                            e1 = ep.tile([P, 2 * TQB], f32r, tag="e")
                            nc.scalar.activation(e0[:], s0[:], AF.Exp, scale=SCALE)
                            nc.scalar.activation(e1[:], s1[:], AF.Exp, scale=SCALE)
                            for j in range(2):
                                tk = tkk + j
                                jd = tk - 4 * tqb  # diagonal strip index
                                if jd >= 0:
                                    # cols < 128*jd are fully masked; the
                                    # [128*jd, 128*(jd+1)) strip is triangular
                                    for e in (e0, e1):
                                        if jd > 0:
                                            nc.vector.memset(
                                                e[:, j * TQB : j * TQB + jd * P].bitcast(
                                                    f32
                                                ),
                                                0.0,
                                            )
                                        st = j * TQB + jd * P
                                        nc.vector.tensor_mul(
                                            e[:, st : st + P], e[:, st : st + P], tri
                                        )
                            for j in range(2):
                                tk = tkk + j
                                nc.tensor.matmul(
                                    y0[:],
                                    v[:, tk, h0, :],
                                    e0[:, ts(j, TQB)],
                                    start=(tk == 0),
                                    stop=(tk == ntk - 1),
                                )
                                nc.tensor.matmul(
                                    y1[:],
                                    v[:, tk, h1, :],
                                    e1[:, ts(j, TQB)],
                                    start=(tk == 0),
                                    stop=(tk == ntk - 1),
                                )
                        den0 = rp.tile([1, TQB], f32, tag="den")
                        den1 = rp.tile([1, TQB], f32, tag="den")
                        nc.vector.tensor_copy(den0[:], y0[64:65, :])
                        nc.vector.tensor_copy(den1[:], y1[64:65, :])
                        rec0 = rp.tile([1, TQB], f32, tag="rec")
                        rec1 = rp.tile([1, TQB], f32, tag="rec")
                        nc.vector.reciprocal_approx_fast(rec0[:], den0[:])
                        nc.vector.reciprocal_approx_fast(rec1[:], den1[:])
                        rb0 = rp.tile([D, TQB], f32, tag="rb")
                        rb1 = rp.tile([D, TQB], f32, tag="rb")
                        nc.gpsimd.partition_broadcast(rb0[:], rec0[:])
                        nc.gpsimd.partition_broadcast(rb1[:], rec1[:])
                        nc.vector.tensor_mul(
                            yt[0:64, hp, ts(tqb, TQB)], y0[0:64, :], rb0[:]
                        )
                        nc.vector.tensor_mul(
                            yt[64:128, hp, ts(tqb, TQB)], y1[0:64, :], rb1[:]
                        )

                    # ---- partial output projection for this tq block ----
                    for mt in range(4 * tqb, 4 * tqb + 4):
                        for nb in range(2):
                            pj = ps.tile([P, 512], f32, tag="mm512")
                            for kc in range(2):
                                nc.tensor.matmul(
                                    pj[:],
                                    yt[:, kc, ts(mt, P)],
                                    wp[:, kc, ts(nb, 512)],
                                    start=(kc == 0),
                                    stop=(kc == 1),
                                )
                            ot = op.tile([P, 512], f32, tag="ot")
                            nc.vector.tensor_copy(ot[:], pj[:])
                            nc.sync.dma_start(
                                out_d.ap()[ts(mt, P), ts(nb, 512)], ot[:]
                            )

    nc.compile()
    return nc


def _get_nc():
    global _NC
    if _NC is None:
        _NC = _build()
    return _NC


def _shard(x, W_qkv, b_qkv, W_proj, b_proj):
    x = np.ascontiguousarray(np.asarray(x, dtype=np.float32))
    W_qkv = np.ascontiguousarray(np.asarray(W_qkv, dtype=np.float32))
    b_qkv = np.ascontiguousarray(np.asarray(b_qkv, dtype=np.float32))
    W_proj = np.ascontiguousarray(np.asarray(W_proj, dtype=np.float32))
    in_maps = []
    for core in range(8):
        b, g = core // 4, core % 4
        cs = slice(g * HC, (g + 1) * HC)
        in_maps.append(
            {
                "x": np.ascontiguousarray(x[b]),
                "wq": np.ascontiguousarray(W_qkv[:, 0 * C :][:, cs]),
                "wk": np.ascontiguousarray(W_qkv[:, 1 * C :][:, cs]),
                "wv": np.ascontiguousarray(W_qkv[:, 2 * C :][:, cs]),
                "wp": np.ascontiguousarray(W_proj[cs, :]),
                "bq": np.ascontiguousarray(b_qkv[0 * C :][cs]),
                "bk": np.ascontiguousarray(b_qkv[1 * C :][cs]),
                "bv": np.ascontiguousarray(b_qkv[2 * C :][cs]),
            }
        )
    return in_maps


def _gather(results, b_proj):
    b_proj = np.asarray(b_proj, dtype=np.float32)
    y = np.empty((2, T, C), dtype=np.float32)
    for b in range(2):
        acc = results[4 * b]["out"].astype(np.float32).copy()
        for g in range(1, 4):
            acc += results[4 * b + g]["out"]
        y[b] = acc + b_proj
    return y


def kernel(x, W_qkv, b_qkv, W_proj, b_proj):
    nc = _get_nc()
    in_maps = _shard(x, W_qkv, b_qkv, W_proj, b_proj)
    res = bass_utils.run_bass_kernel_spmd(nc, in_maps, core_ids=list(range(8)))
    return _gather(res.results, b_proj)
